# revision 7
# baseline (speedup 1.0000x reference)
"""Causal self-attention Trainium2 kernel (8-core SPMD).

Problem: nn_CausalSelfAttention_13932873908859
  x [2, 2048, 1024] f32; Wqkv [1024, 3072]; bqkv [3072]; Wout [1024, 1024];
  bout [1024]; 16 heads x 64 dim.

Sharding (8 cores): core c -> batch b = c // 4, head group g = c % 4
(heads 4g..4g+4).  Each core computes its 4 heads' qkv projection,
causal attention, and the partial out-projection for its head rows of
Wout.  Host sums the 4 partials per batch and adds bout.

Per-core pipeline (matmuls in float32r: full-rate fp32 on PE; every
fp32r operand is produced by a compute op so it is rounded to the
fp32r storage format, per the BIR verifier rule):
  1. x [S,D] -> xT [D,S] via PE transposes in plain fp32 (fp32 has no
     DMA transpose); the PSUM->SBUF copy rounds into fp32r.
  2. qT/kT [256, S] = (Wq/Wk)^T x^T + b (d-major so scores contract
     over head_dim on partitions); v [S, 256] natural (bias via K=1
     ones outer-product matmul folded into the PSUM accumulation).
  3. Per (head pair, q-block of 512): scores^T [keys,q] = kT^T qT per
     128-key chunk (causal: only chunks <= diagonal; even/odd heads
     sit at partition bases 0/64 so their K=64 score matmuls pack
     into disjoint PE row groups), exp on ScalarE with scale=1/8 and
     shift -3 folded in (max |score/8| ~ 2.6 so no row-max pass is
     needed; the shift cancels in the softmax ratio), triangular mask
     on the diagonal chunk, then ctxT accumulation ctxT_aug [65, q] +=
     v_aug^T pT with v_aug carrying a ones column so row 64
     accumulates the softmax denominator Z.
  4. ctxT rows are scaled by 1/Z (Z broadcast across partitions via a
     K=1 ones matmul) and fed as lhsT into the out-projection.
"""

import sys

sys.path.insert(0, "/opt/trn_rl_repo")

import numpy as np

B, S, D = 2, 2048, 1024
H, HD = 16, 64
NCORES = 8
HPC = 4  # heads per core
P = 128
NKC = S // P  # 16 key/seq chunks of 128
QB = 512  # q-block width
NQB = S // QB  # 4 q blocks
DCH = D // P  # 8 d chunks
LOC = HPC * HD  # 256 local feature cols per core
EXP_SHIFT = 3.0

_BUILT = {}


def _build_module():
    import concourse.bacc as bacc
    import concourse.mybir as mybir
    import concourse.tile as tile
    from concourse.masks import make_identity, make_upper_triangular

    dt = mybir.dt
    f32 = dt.float32
    f32r = dt.float32r
    Exp = mybir.ActivationFunctionType.Exp
    mult = mybir.AluOpType.mult
    add = mybir.AluOpType.add

    nc = bacc.Bacc("TRN2", target_bir_lowering=False)

    xb = nc.dram_tensor("xb", [S, D], f32, kind="ExternalInput")
    wqkv = nc.dram_tensor("wqkv", [D, 3 * LOC], f32, kind="ExternalInput")
    bqkv = nc.dram_tensor("bqkv", [3 * LOC], f32, kind="ExternalInput")
    wout = nc.dram_tensor("wout", [LOC, D], f32, kind="ExternalInput")
    outp = nc.dram_tensor("outp", [S, D], f32, kind="ExternalOutput")

    with tile.TileContext(nc) as tc:
        with (
            tc.tile_pool(name="persist", bufs=1) as persist,
            tc.tile_pool(name="psmall", bufs=2) as psmall,
            tc.tile_pool(name="ppool", bufs=4) as ppool,
            tc.tile_pool(name="outsb", bufs=2) as outsb,
        ):
            # ---- constants ----
            ident = persist.tile([P, P], f32, tag="ident")
            make_identity(nc, ident[:, :])
            tri_f = persist.tile([P, P], f32, tag="trif")
            # tri[k, q] = 1 where q >= k else 0 (upper incl. diagonal)
            make_upper_triangular(nc, tri_f[:, :], val=1.0, diag=True)
            tri = persist.tile([P, P], f32r, tag="tri")
            nc.vector.tensor_copy(out=tri[:, :], in_=tri_f[:, :])
            ones_f = persist.tile([P, 1], f32, tag="onesf")
            nc.gpsimd.memset(ones_f[:, :], 1.0)
            ones_row = persist.tile([1, P], f32r, tag="ones")
            nc.vector.tensor_copy(
                out=ones_row[:, :], in_=ones_f[0:1, 0:1].to_broadcast((1, P))
            )
            zeros_f = persist.tile([P, 3 * P], f32, tag="zerof")
            nc.gpsimd.memset(zeros_f[:, :], 0.0)
            nbias = persist.tile([P, 1], f32, tag="nbias")
            nc.gpsimd.memset(nbias[:, :], -EXP_SHIFT)

            bq_col = persist.tile([P, 2], f32, tag="bq")
            nc.sync.dma_start(bq_col[:, :], bqkv[0:LOC].rearrange("(c p) -> p c", p=P))
            bk_col = persist.tile([P, 2], f32, tag="bk")
            nc.sync.dma_start(
                bk_col[:, :], bqkv[LOC : 2 * LOC].rearrange("(c p) -> p c", p=P)
            )
            bv_stage = persist.tile([1, LOC], f32, tag="bvs")
            nc.sync.dma_start(bv_stage[:, :], bqkv[2 * LOC : 3 * LOC].unsqueeze(0))
            bv_row = persist.tile([1, LOC], f32r, tag="bv")
            nc.vector.tensor_copy(out=bv_row[:, :], in_=bv_stage[:, :])

            # weights: DMA chunkwise to f32 staging, round into f32r tiles
            wqkv_r = persist.tile([P, DCH, 3 * LOC], f32r, tag="wqkv")
            wout_r = persist.tile([P, 2, D], f32r, tag="wout")

            # ---- persistent activations ----
            xT = persist.tile([P, DCH, S], f32r, tag="xT")
            qT = persist.tile([P, 2, S], f32r, tag="qT")
            kT = persist.tile([P, 2, S], f32r, tag="kT")
            v_aug = persist.tile([P, NKC, HPC, HD + 1], f32r, tag="vaug")
            nc.vector.tensor_copy(
                out=v_aug[:, :, :, HD : HD + 1],
                in_=ones_f[:, 0:1].to_broadcast((P, NKC, HPC, 1)),
            )
            ctxT = persist.tile([P, 2, S], f32r, tag="ctxT")

            # ---- phase 0/1: load + transpose x, qkv projection ----
            with (
                tc.tile_pool(name="ph1ps", bufs=2, space="PSUM") as ph1ps,
                tc.tile_pool(name="xstage", bufs=3) as xstage,
            ):
                for dc in range(DCH):
                    ws = xstage.tile([P, D], f32, tag="xs", name=f"ws_{dc}")[:, : 3 * LOC]
                    nc.sync.dma_start(
                        ws[:, :], wqkv[dc * P : (dc + 1) * P, :]
                    )
                    nc.vector.tensor_copy(out=wqkv_r[:, dc, :], in_=ws[:, :])
                for f in range(2):
                    wo = xstage.tile([P, D], f32, tag="xs", name=f"wo_{f}")
                    nc.sync.dma_start(wo[:, :], wout[f * P : (f + 1) * P, :])
                    nc.vector.tensor_copy(out=wout_r[:, f, :], in_=wo[:, :])

                for sc in range(NKC):
                    xs = xstage.tile([P, D], f32, tag="xs")
                    nc.sync.dma_start(xs[:, :], xb[sc * P : (sc + 1) * P, :])
                    for dc in range(DCH):
                        tp = ph1ps.tile([P, P], f32, tag="tp")
                        nc.tensor.transpose(
                            tp[:, :], xs[:, dc * P : (dc + 1) * P], ident[:, :]
                        )
                        nc.any.tensor_copy(
                            out=xT[:, dc, sc * P : (sc + 1) * P], in_=tp[:, :]
                        )

                # qT / kT: [256, S] d-major, via lhsT = W chunk, rhs = xT
                for which, wofs, bcol, dest in (
                    ("q", 0, bq_col, qT),
                    ("k", LOC, bk_col, kT),
                ):
                    for c in range(2):
                        for sb in range(NQB):
                            ps = ph1ps.tile([P, QB], f32, tag="pqk")
                            for kc in range(DCH):
                                nc.tensor.matmul(
                                    ps[:, :],
                                    wqkv_r[:, kc, wofs + c * P : wofs + (c + 1) * P],
                                    xT[:, kc, sb * QB : (sb + 1) * QB],
                                    start=(kc == 0),
                                    stop=(kc == DCH - 1),
                                )
                            nc.vector.tensor_tensor(
                                dest[:, c, sb * QB : (sb + 1) * QB],
                                ps[:, :],
                                bcol[:, c : c + 1].to_broadcast((P, QB)),
                                add,
                            )

                # v: natural [S, 256], bias folded in as K=1 ones x bv_row
                for sc in range(NKC):
                    ps = ph1ps.tile([P, LOC], f32, tag="pv")
                    for kc in range(DCH):
                        nc.tensor.matmul(
                            ps[:, :],
                            xT[:, kc, sc * P : (sc + 1) * P],
                            wqkv_r[:, kc, 2 * LOC : 3 * LOC],
                            start=(kc == 0),
                            stop=False,
                        )
                    nc.tensor.matmul(
                        ps[:, :],
                        ones_row[0:1, 0:P],
                        bv_row[0:1, :],
                        start=False,
                        stop=True,
                    )
                    nc.any.tensor_copy(
                        out=v_aug[:, sc, :, 0:HD],
                        in_=ps[:, :].rearrange("p (h d) -> p h d", h=HPC),
                    )

            # ---- phase 2: attention, transposed-scores scheme ----
            with (
                tc.tile_pool(name="scps", bufs=3, space="PSUM") as scps,
                tc.tile_pool(name="ctxps", bufs=2, space="PSUM") as ctxps,
                tc.tile_pool(name="rbps", bufs=2, space="PSUM") as rbps,
            ):
                for hpair in range(2):
                    for j in range(NQB):
                        cps = {}
                        for hh in range(2):
                            h = 2 * hpair + hh
                            cps[h] = ctxps.tile(
                                [HD + 1, QB], f32, tag="ctx", name=f"ctx_{h}"
                            )
                        nkc = 4 * j + 4  # key chunks 0..4j+3
                        for kc in range(nkc):
                            doff = kc * P - j * QB
                            off = max(0, doff)
                            sps = {}
                            pts = {}
                            # both heads' score matmuls back-to-back: they
                            # sit at PE row groups 0/64 and run concurrently
                            for hh in range(2):
                                h = 2 * hpair + hh
                                rlo = 64 * hh
                                rhi = rlo + HD
                                c = h // 2
                                sp = scps.tile([P, QB], f32, tag="sc", name=f"sc_{h}")
                                nc.tensor.matmul(
                                    sp[:, off:QB],
                                    kT[rlo:rhi, c, kc * P : (kc + 1) * P],
                                    qT[rlo:rhi, c, j * QB + off : (j + 1) * QB],
                                    start=True,
                                    stop=True,
                                )
                                sps[h] = sp
                            for hh in range(2):
                                h = 2 * hpair + hh
                                pt = ppool.tile([P, QB], f32r, tag="pt", name=f"pt_{h}")
                                if doff > 0:
                                    nc.vector.tensor_copy(
                                        out=pt[:, 0:doff], in_=zeros_f[:, 0:doff]
                                    )
                                nc.scalar.activation(
                                    pt[:, off:QB],
                                    sps[h][:, off:QB],
                                    Exp,
                                    bias=nbias[:, :],
                                    scale=0.125,
                                )
                                if doff >= 0:
                                    nc.vector.tensor_tensor(
                                        pt[:, doff : doff + P],
                                        pt[:, doff : doff + P],
                                        tri[:, :],
                                        mult,
                                    )
                                pts[h] = pt
                            for hh in range(2):
                                h = 2 * hpair + hh
                                nc.tensor.matmul(
                                    cps[h][:, :],
                                    v_aug[:, kc, h, :],
                                    pts[h][:, :],
                                    start=(kc == 0),
                                    stop=(kc == nkc - 1),
                                )
                        for hh in range(2):
                            h = 2 * hpair + hh
                            rz_f = psmall.tile([1, QB], f32, tag="rzf")
                            nc.vector.reciprocal(rz_f[:, :], cps[h][HD : HD + 1, :])
                            rz = psmall.tile([1, QB], f32r, tag="rz")
                            nc.vector.tensor_copy(out=rz[:, :], in_=rz_f[:, :])
                            rb = rbps.tile([HD, QB], f32, tag="rb")
                            nc.tensor.matmul(
                                rb[:, :],
                                ones_row[0:1, 0:HD],
                                rz[:, :],
                                start=True,
                                stop=True,
                            )
                            rbs = psmall.tile([HD, QB], f32, tag="rbs")
                            nc.any.tensor_copy(out=rbs[:, :], in_=rb[:, :])
                            nc.vector.tensor_tensor(
                                ctxT[
                                    64 * hh : 64 * hh + HD,
                                    h // 2,
                                    j * QB : (j + 1) * QB,
                                ],
                                cps[h][0:HD, :],
                                rbs[:, :],
                                mult,
                            )

            # ---- phase 3: out projection ----
            with tc.tile_pool(name="ops", bufs=3, space="PSUM") as ops:
                for qc in range(NKC):
                    osb = outsb.tile([P, D], f32, tag="osb")
                    for nb in range(2):
                        po = ops.tile([P, QB], f32, tag="po")
                        for f in range(2):
                            nc.tensor.matmul(
                                po[:, :],
                                ctxT[:, f, qc * P : (qc + 1) * P],
                                wout_r[:, f, nb * QB : (nb + 1) * QB],
                                start=(f == 0),
                                stop=(f == 1),
                            )
                        nc.any.tensor_copy(
                            out=osb[:, nb * QB : (nb + 1) * QB], in_=po[:, :]
                        )
                    nc.sync.dma_start(outp[qc * P : (qc + 1) * P, :], osb[:, :])

    nc.compile()
    return nc


def _get_module():
    if "nc" not in _BUILT:
        _BUILT["nc"] = _build_module()
    return _BUILT["nc"]


def _shard_inputs(x, Wqkv, bqkv, Wout):
    """Per-core input dicts: core c -> batch c//4, heads [4*(c%4), 4*(c%4)+4)."""
    x = np.ascontiguousarray(np.asarray(x, dtype=np.float32))
    Wqkv = np.asarray(Wqkv, dtype=np.float32)
    bqkv = np.asarray(bqkv, dtype=np.float32)
    Wout = np.asarray(Wout, dtype=np.float32)
    in_maps = []
    for c in range(NCORES):
        b = c // 4
        g = c % 4
        lo = g * LOC
        hi = lo + LOC
        cols = np.concatenate(
            [
                np.arange(lo, hi),
                np.arange(D + lo, D + hi),
                np.arange(2 * D + lo, 2 * D + hi),
            ]
        )
        in_maps.append(
            {
                "xb": np.ascontiguousarray(x[b]),
                "wqkv": np.ascontiguousarray(Wqkv[:, cols]),
                "bqkv": np.ascontiguousarray(bqkv[cols]),
                "wout": np.ascontiguousarray(Wout[lo:hi, :]),
            }
        )
    return in_maps


LAST_RESULTS = None


def kernel(x, Wqkv, bqkv, Wout, bout):
    global LAST_RESULTS
    from concourse.bass_utils import run_bass_kernel_spmd

    nc = _get_module()
    in_maps = _shard_inputs(x, Wqkv, bqkv, Wout)
    res = run_bass_kernel_spmd(nc, in_maps, core_ids=list(range(NCORES)))
    LAST_RESULTS = res
    bout = np.asarray(bout, dtype=np.float32)
    out = np.zeros((B, S, D), dtype=np.float32)
    for c in range(NCORES):
        out[c // 4] += res.results[c]["outp"]
    out += bout[None, None, :]
    return out
